# revision 20
# baseline (speedup 1.0000x reference)
"""Trainium2 Bass kernel for nn_AutoregressiveBisectionInverter (v11).

Closed-form cubic root per autoregressive step: solve v^3+v = nd via
v = (t^(1/3) - t^(-1/3))/sqrt(3), t = z + sqrt(z^2+1), z = (3sqrt3/2)nd;
x_k = sat_k * v. With E' = sat*e^(ln(t)/3) (sat folded into Exp's bias),
x_k = E' - sat^2/E'.

v11 structure (vs v10): the prefix-dot machinery (paired dots + special
chains + cbfix) is replaced by a rank-1 accumulator update: P[:, k+2:]
+= W[k+2:, k]*x_k, one wide scalar_tensor_tensor per step. tanh_k reads
bias directly from P[:, k] (P initialized to b via DMA), with the final
W[k,k-1]*x_{k-1} term folded into tanh's input scale. z moves to DVE for
every step. The update is emitted between z_{k+1} and rcp_{k+1} so its
~90ns engine time hides in the DVE's z->rcp dependency gap instead of
blocking the critical path.

Per step: ACT {tanh, Square, Sqrt, Ln, Exp} = 5 ops (285ns issue; the
settled steady-state period = the ACT sequencer floor), DVE {z, upd,
rcp, x} = 4 ops (280ns issue). Semaphore edges (one wait per op; every
adjacent same-engine RAW gets a completion-sem edge for the write-ack
window; everything else is covered transitively):
  tanh_k   [w sV=d_x[k-1]]  (covers upd_{k-2}'s P write via DVE order)
  Square_k [w sV=d_z[k]]    (covers tanh_k transitively + z RAW for Ln)
  Sqrt/Ln/Exp [w sA=prev]   (adjacent-op ack edges)
  z_k [w sA=a_tanh[k]], rcp_k [w sA=a_e[k]], x_k [w sV=d_rcp[k]],
  upd_k [w sV=d_x[k]]       (satisfied by exec time; formal ack edge)
Input is 3 DMAs: DMA1 is padded to exactly 512 B/partition (descriptors
below 512 B pay a 2x latency multiplier, so the padded span copies
faster than the bare 452 B) and carries everything steps 0-2 touch;
gate memsets on the DVE cover the later spans. Output is one SP DMA
gated on x_31 (its 650ns SEQ config runs early and spans the wait, so
only HWDGE+DGE+copy+sem trail the last step).
"""

import numpy as np

B, D = 1024, 32
NCORES = 8
ROWS = B // NCORES  # 128 rows per core == SBUF partitions


def _softplus64(x):
    x = x.astype(np.float64)
    return np.log1p(np.exp(-np.abs(x))) + np.maximum(x, 0)


def build(y, W, s, b):
    """Build the SPMD Bass program; returns (nc, in_maps)."""
    from contextlib import ExitStack
    import concourse.bass as bass
    from concourse import mybir

    f32 = mybir.dt.float32
    Alu = mybir.AluOpType
    Act = mybir.ActivationFunctionType

    y = np.ascontiguousarray(np.asarray(y), dtype=np.float32)
    W64 = np.asarray(W, dtype=np.float64)
    s64 = np.asarray(s, dtype=np.float64)
    b64 = np.asarray(b, dtype=np.float64)

    # ---- host precompute (elementwise input normalization only) ----
    abar = 10.0 * _softplus64(s64)
    sqrt_abar = np.sqrt(abar)
    CC = 3.0 * np.sqrt(3.0) / 2.0
    kz = (CC * 10.0 * abar ** -1.5).astype(np.float32)
    Yz = (CC * 10.0 * y.astype(np.float64) * abar[None, :] ** -1.5).astype(np.float32)
    sat64 = sqrt_abar / np.sqrt(3.0)
    lnsat = np.log(sat64).astype(np.float32)
    sat2 = (sat64 * sat64).astype(np.float32)
    Wq = W64.astype(np.float32)
    c0 = float(-kz[0] * np.tanh(b64[0]))

    # hdr columns (single SBUF tensor; three DMA spans):
    #   DMA1: yzA (NA) | lnA (NA) | c0 (1) | P-init = b bcast (D) | WT_0 | WT_1
    #   DMA2: WT_2..WT_9
    #   DMA3: yzB (D-NA) | lnB (D-NA) | WT_10..WT_29
    NA = 10
    wt_w = [D - 2 - j for j in range(D - 2)]  # widths 30..1 for j=0..29
    wt_off = {}
    C0COL = 2 * NA
    PCOL = C0COL + 1
    c = PCOL + D
    for j in (0, 1):
        wt_off[j] = c
        c += wt_w[j]
    for j in range(2, 10):
        wt_off[j] = c
        c += wt_w[j]
    C2 = c
    # DMA1 ends at 128 cols = 512 B/partition: descriptors >= 512 B avoid the
    # 2x small-transfer latency multiplier, so the padded span copies FASTER
    # than the bare 112 cols. The overhang into WT_2.. is covered by g2.
    C1 = 128
    assert C1 >= wt_off[2] and C1 <= C2
    YZB = c
    LNB = YZB + (D - NA)
    c = LNB + (D - NA)
    for j in range(10, D - 2):
        wt_off[j] = c
        c += wt_w[j]
    HW = c

    def hdr_np_for(yzc_):
        h = np.zeros((ROWS, HW), np.float32)
        h[:, 0:NA] = yzc_[:, 0:NA]
        h[:, NA:2 * NA] = lnsat[None, 0:NA]
        h[:, C0COL] = c0
        h[:, PCOL:PCOL + D] = b64.astype(np.float32)[None, :]
        h[:, YZB:YZB + (D - NA)] = yzc_[:, NA:]
        h[:, LNB:LNB + (D - NA)] = lnsat[None, NA:]
        for j in range(D - 2):
            h[:, wt_off[j]:wt_off[j] + wt_w[j]] = Wq[j + 2:, j][None, :]
        return np.ascontiguousarray(h)

    nc = bass.Bass()
    hd_d = nc.dram_tensor("hdr", [ROWS, HW], f32, kind="ExternalInput")
    xo_d = nc.dram_tensor("xout", [ROWS, D], f32, kind="ExternalOutput")

    with ExitStack() as ctx:
        hdr = ctx.enter_context(nc.sbuf_tensor([ROWS, HW], f32))
        vx = ctx.enter_context(nc.sbuf_tensor([ROWS, D], f32))
        tt = ctx.enter_context(nc.sbuf_tensor([ROWS, 1], f32))
        z2 = ctx.enter_context(nc.sbuf_tensor([ROWS, 1], f32))
        rr = ctx.enter_context(nc.sbuf_tensor([ROWS, 1], f32))
        ll = ctx.enter_context(nc.sbuf_tensor([ROWS, 1], f32))
        ee = ctx.enter_context(nc.sbuf_tensor([ROWS, 1], f32))
        rcp = ctx.enter_context(nc.sbuf_tensor([ROWS, 1], f32))
        zz = ctx.enter_context(nc.sbuf_tensor([ROWS, 1], f32))
        scr = ctx.enter_context(nc.sbuf_tensor([ROWS, 1], f32))
        s_dma = ctx.enter_context(nc.semaphore("s_dma"))
        sA = ctx.enter_context(nc.semaphore("sA"))
        sV = ctx.enter_context(nc.semaphore("sV"))
        s_out = ctx.enter_context(nc.semaphore("s_out"))
        block = ctx.enter_context(nc.Block(no_gpsimd_drain=True))

        def yzc(k):
            col = k if k < NA else YZB + (k - NA)
            return hdr[:, col:col + 1]

        def lnc(k):
            col = NA + k if k < NA else LNB + (k - NA)
            return hdr[:, col:col + 1]

        c0c = hdr[:, C0COL:C0COL + 1]
        P = hdr[:, PCOL:PCOL + D]  # written in place by updates

        # ---- pre-pass: semaphore-count landmarks ----
        a_tanh, a_sq, a_sqrt, a_ln, a_e = {}, {}, {}, {}, {}
        pa = 0
        for k in range(D):
            if k >= 1:
                pa += 1
                a_tanh[k] = pa
            pa += 1
            a_sq[k] = pa
            pa += 1
            a_sqrt[k] = pa
            pa += 1
            a_ln[k] = pa
            pa += 1
            a_e[k] = pa

        # DVE order per step k: [g3 at k=10] z_k | upd_{k-1} | rcp_k | x_k
        # upd_{k-1} executes in the z->rcp gap (DVE idles ~135ns there
        # waiting on the ACT chain), keeping its wide op off the critical
        # path. [g2 before upd_2, i.e. in step 3's slot.]
        d_z, d_rcp, d_x, d_upd = {}, {}, {}, {}
        pd = 0
        for k in range(D):
            if k == 10:
                pd += 1  # g3: DMA3 gate (yzB + lnB + WT_10..)
            pd += 1
            d_z[k] = pd
            if 1 <= k <= D - 2:
                if k == 3:
                    pd += 1  # g2: DMA2 gate (WT_2..WT_9)
                pd += 1
                d_upd[k - 1] = pd
            pd += 1
            d_rcp[k] = pd
            pd += 1
            d_x[k] = pd

        @block.scalar
        def _(scalar):
            for k in range(D):
                if k >= 1:
                    nc.scalar.activation(
                        out=tt[:, :], in_=vx[:, k - 1:k], func=Act.Tanh,
                        bias=P[:, k:k + 1],
                        scale=float(Wq[k, k - 1]))._wait_ge(
                            sV, d_x[k - 1]).then_inc(sA, 1)
                # z2 = (scale*T + yz)^2 ; k=0: z_0 = yz_0 + c0
                if k == 0:
                    nc.scalar.activation(
                        out=z2[:, :], in_=yzc(0), func=Act.Square,
                        bias=c0c[:, :], scale=1.0)._wait_ge(
                            sV, d_z[0]).then_inc(sA, 1)
                else:
                    nc.scalar.activation(
                        out=z2[:, :], in_=tt[:, :], func=Act.Square,
                        bias=yzc(k), scale=float(-kz[k]))._wait_ge(
                            sV, d_z[k]).then_inc(sA, 1)
                nc.scalar.activation(
                    out=rr[:, :], in_=z2[:, :], func=Act.Sqrt,
                    bias=1.0, scale=1.0)._wait_ge(
                        sA, a_sq[k]).then_inc(sA, 1)
                nc.scalar.activation(
                    out=ll[:, :], in_=rr[:, :], func=Act.Ln, bias=zz[:, :],
                    scale=1.0)._wait_ge(sA, a_sqrt[k]).then_inc(sA, 1)
                nc.scalar.activation(
                    out=ee[:, :], in_=ll[:, :], func=Act.Exp,
                    bias=lnc(k), scale=float(1.0 / 3.0))._wait_ge(
                        sA, a_ln[k]).then_inc(sA, 1)

        @block.vector
        def _(vector):
            for k in range(D):
                if k == 10:
                    # DMA3 gate: z/Square k>=10 (yzB) and upd k>=10 (WT)
                    nc.vector.memset(scr[:, 0:1], 0.0)._wait_ge(
                        s_dma, 48).then_inc(sV, 1)
                # z_k = -kz*T + yz  (k=0: yz_0 + c0)
                if k == 0:
                    nc.vector.tensor_scalar(
                        out=zz[:, :], in0=yzc(0), scalar1=c0,
                        scalar2=None, op0=Alu.add)._wait_ge(
                            s_dma, 16).then_inc(sV, 1)
                else:
                    nc.vector.tensor_scalar(
                        out=zz[:, :], in0=tt[:, :],
                        scalar1=float(-kz[k]), scalar2=yzc(k),
                        op0=Alu.mult, op1=Alu.add)._wait_ge(
                            sA, a_tanh[k]).then_inc(sV, 1)
                # P[:, j+2:] += W[j+2:, j] * x_j  for j = k-1 (in z->rcp gap)
                if 1 <= k <= D - 2:
                    j = k - 1
                    if k == 3:
                        # DMA2 gate: WT_2..WT_9
                        nc.vector.memset(scr[:, 0:1], 0.0)._wait_ge(
                            s_dma, 32).then_inc(sV, 1)
                    nc.vector.scalar_tensor_tensor(
                        out=P[:, j + 2:D],
                        in0=hdr[:, wt_off[j]:wt_off[j] + wt_w[j]],
                        scalar=vx[:, j:j + 1],
                        in1=P[:, j + 2:D],
                        op0=Alu.mult, op1=Alu.add)._wait_ge(
                            sV, d_x[j]).then_inc(sV, 1)
                nc.vector.reciprocal(out=rcp[:, :], in_=ee[:, :])._wait_ge(
                    sA, a_e[k]).then_inc(sV, 1)
                # x_k = -sat^2*rcp + E'
                nc.vector.tensor_scalar(
                    out=vx[:, k:k + 1], in0=rcp[:, :],
                    scalar1=float(-sat2[k]), scalar2=ee[:, 0:1],
                    op0=Alu.mult, op1=Alu.add)._wait_ge(
                        sV, d_rcp[k]).then_inc(sV, 1)

        @block.sync
        def _(sync):
            sync.dma_start(out=hdr[:, 0:C1],
                           in_=hd_d[:, 0:C1]).then_inc(s_dma, 16)
            sync.dma_start(out=hdr[:, C1:C2],
                           in_=hd_d[:, C1:C2]).then_inc(s_dma, 16)
            sync.dma_start(out=hdr[:, C2:HW],
                           in_=hd_d[:, C2:HW]).then_inc(s_dma, 16)
            sync.dma_start(out=xo_d[:, :], in_=vx[:, :])._wait_ge(
                sV, d_x[D - 1]).then_inc(s_out, 16)
            sync.wait_ge(s_dma, 48)
            sync.wait_ge(s_out, 16)

    in_maps = []
    for c_ in range(NCORES):
        yzc_ = Yz[c_ * ROWS:(c_ + 1) * ROWS]
        in_maps.append({"hdr": hdr_np_for(yzc_)})
    return nc, in_maps


def kernel(y, W, s, b):
    from concourse.bass_utils import run_bass_kernel_spmd

    nc, in_maps = build(y, W, s, b)
    res = run_bass_kernel_spmd(nc, in_maps, list(range(NCORES))).results
    X = np.concatenate([res[c]["xout"] for c in range(NCORES)], axis=0)
    return X.astype(np.float32)


if __name__ == "__main__":
    rng = np.random.default_rng(0)
    y = rng.standard_normal((B, D)).astype(np.float32)
    W = np.tril(rng.standard_normal((32, 32)), -1).astype(np.float32) * 0.5
    s = rng.standard_normal(D).astype(np.float32)
    b = rng.standard_normal(D).astype(np.float32)
    X = kernel(y=y, W=W, s=s, b=b)
    print("out", X.shape, X.dtype, X[0, :4])


# revision 24
# speedup vs baseline: 1.0006x; 1.0006x over previous
"""Trainium2 Bass kernel for nn_AutoregressiveBisectionInverter (v11).

Closed-form cubic root per autoregressive step: solve v^3+v = nd via
v = (t^(1/3) - t^(-1/3))/sqrt(3), t = z + sqrt(z^2+1), z = (3sqrt3/2)nd;
x_k = sat_k * v. With E' = sat*e^(ln(t)/3) (sat folded into Exp's bias),
x_k = E' - sat^2/E'.

v11 structure (vs v10): the prefix-dot machinery (paired dots + special
chains + cbfix) is replaced by a rank-1 accumulator update: P[:, k+2:]
+= W[k+2:, k]*x_k, one wide scalar_tensor_tensor per step. tanh_k reads
bias directly from P[:, k] (P initialized to b via DMA), with the final
W[k,k-1]*x_{k-1} term folded into tanh's input scale. z moves to DVE for
every step. The update is emitted between z_{k+1} and rcp_{k+1} so its
~90ns engine time hides in the DVE's z->rcp dependency gap instead of
blocking the critical path.

Per step: ACT {tanh, Square, Sqrt, Ln, Exp} = 5 ops (285ns issue; the
settled steady-state period = the ACT sequencer floor), DVE {z, upd,
rcp, x} = 4 ops (280ns issue). Semaphore edges (one wait per op; every
adjacent same-engine RAW gets a completion-sem edge for the write-ack
window; everything else is covered transitively):
  tanh_k   [w sV=d_x[k-1]]  (covers upd_{k-2}'s P write via DVE order)
  Square_k [w sV=d_z[k]]    (covers tanh_k transitively + z RAW for Ln)
  Sqrt/Ln/Exp [w sA=prev]   (adjacent-op ack edges)
  z_k [w sA=a_tanh[k]], rcp_k [w sA=a_e[k]], x_k [w sV=d_rcp[k]],
  upd_k [w sV=d_x[k]]       (satisfied by exec time; formal ack edge)
Input is 3 DMAs: DMA1 is padded to exactly 512 B/partition (descriptors
below 512 B pay a 2x latency multiplier, so the padded span copies
faster than the bare 452 B) and carries everything steps 0-2 touch;
gate memsets on the DVE cover the later spans. Output is one SP DMA
gated on x_31 (its 650ns SEQ config runs early and spans the wait, so
only HWDGE+DGE+copy+sem trail the last step).
"""

import numpy as np

B, D = 1024, 32
NCORES = 8
ROWS = B // NCORES  # 128 rows per core == SBUF partitions


def _softplus64(x):
    x = x.astype(np.float64)
    return np.log1p(np.exp(-np.abs(x))) + np.maximum(x, 0)


def build(y, W, s, b):
    """Build the SPMD Bass program; returns (nc, in_maps)."""
    from contextlib import ExitStack
    import concourse.bass as bass
    from concourse import mybir

    f32 = mybir.dt.float32
    Alu = mybir.AluOpType
    Act = mybir.ActivationFunctionType

    y = np.ascontiguousarray(np.asarray(y), dtype=np.float32)
    W64 = np.asarray(W, dtype=np.float64)
    s64 = np.asarray(s, dtype=np.float64)
    b64 = np.asarray(b, dtype=np.float64)

    # ---- host precompute (elementwise input normalization only) ----
    abar = 10.0 * _softplus64(s64)
    sqrt_abar = np.sqrt(abar)
    CC = 3.0 * np.sqrt(3.0) / 2.0
    kz = (CC * 10.0 * abar ** -1.5).astype(np.float32)
    Yz = (CC * 10.0 * y.astype(np.float64) * abar[None, :] ** -1.5).astype(np.float32)
    sat64 = sqrt_abar / np.sqrt(3.0)
    lnsat = np.log(sat64).astype(np.float32)
    sat2 = (sat64 * sat64).astype(np.float32)
    Wq = W64.astype(np.float32)
    c0 = float(-kz[0] * np.tanh(b64[0]))

    # hdr columns (single SBUF tensor; three DMA spans):
    #   DMA1: yzA (NA) | lnA (NA) | c0 (1) | P-init = b bcast (D) | WT_0 | WT_1
    #   DMA2: WT_2..WT_9
    #   DMA3: yzB (D-NA) | lnB (D-NA) | WT_10..WT_29
    NA = 10
    wt_w = [D - 2 - j for j in range(D - 2)]  # widths 30..1 for j=0..29
    wt_off = {}
    C0COL = 2 * NA
    PCOL = C0COL + 1
    c = PCOL + D
    for j in (0, 1):
        wt_off[j] = c
        c += wt_w[j]
    for j in range(2, 10):
        wt_off[j] = c
        c += wt_w[j]
    C2 = c
    # DMA1 ends at 128 cols = 512 B/partition: descriptors >= 512 B avoid the
    # 2x small-transfer latency multiplier, so the padded span copies FASTER
    # than the bare 112 cols. The overhang into WT_2.. is covered by g2.
    C1 = 128
    assert C1 >= wt_off[2] and C1 <= C2
    YZB = c
    LNB = YZB + (D - NA)
    c = LNB + (D - NA)
    for j in range(10, D - 2):
        wt_off[j] = c
        c += wt_w[j]
    HW = c

    def hdr_np_for(yzc_):
        h = np.zeros((ROWS, HW), np.float32)
        h[:, 0:NA] = yzc_[:, 0:NA]
        h[:, NA:2 * NA] = lnsat[None, 0:NA]
        h[:, C0COL] = c0
        h[:, PCOL:PCOL + D] = b64.astype(np.float32)[None, :]
        h[:, YZB:YZB + (D - NA)] = yzc_[:, NA:]
        h[:, LNB:LNB + (D - NA)] = lnsat[None, NA:]
        for j in range(D - 2):
            h[:, wt_off[j]:wt_off[j] + wt_w[j]] = Wq[j + 2:, j][None, :]
        return np.ascontiguousarray(h)

    nc = bass.Bass()
    hd_d = nc.dram_tensor("hdr", [ROWS, HW], f32, kind="ExternalInput")
    xo_d = nc.dram_tensor("xout", [ROWS, D], f32, kind="ExternalOutput")

    with ExitStack() as ctx:
        hdr = ctx.enter_context(nc.sbuf_tensor([ROWS, HW], f32))
        vx = ctx.enter_context(nc.sbuf_tensor([ROWS, D], f32))
        tt = ctx.enter_context(nc.sbuf_tensor([ROWS, 1], f32))
        z2 = ctx.enter_context(nc.sbuf_tensor([ROWS, 1], f32))
        rr = ctx.enter_context(nc.sbuf_tensor([ROWS, 1], f32))
        ll = ctx.enter_context(nc.sbuf_tensor([ROWS, 1], f32))
        ee = ctx.enter_context(nc.sbuf_tensor([ROWS, 1], f32))
        rcp = ctx.enter_context(nc.sbuf_tensor([ROWS, 1], f32))
        zz = ctx.enter_context(nc.sbuf_tensor([ROWS, 1], f32))
        scr = ctx.enter_context(nc.sbuf_tensor([ROWS, 1], f32))
        s_dma = ctx.enter_context(nc.semaphore("s_dma"))
        sA = ctx.enter_context(nc.semaphore("sA"))
        sV = ctx.enter_context(nc.semaphore("sV"))
        s_out = ctx.enter_context(nc.semaphore("s_out"))
        block = ctx.enter_context(nc.Block(no_gpsimd_drain=True))

        def yzc(k):
            col = k if k < NA else YZB + (k - NA)
            return hdr[:, col:col + 1]

        def lnc(k):
            col = NA + k if k < NA else LNB + (k - NA)
            return hdr[:, col:col + 1]

        c0c = hdr[:, C0COL:C0COL + 1]
        P = hdr[:, PCOL:PCOL + D]  # written in place by updates

        # ---- pre-pass: semaphore-count landmarks ----
        a_tanh, a_sq, a_sqrt, a_ln, a_e = {}, {}, {}, {}, {}
        pa = 0
        for k in range(D):
            if k >= 1:
                pa += 1
                a_tanh[k] = pa
            if k != D - 1:
                pa += 1
                a_sq[k] = pa
            pa += 1
            a_sqrt[k] = pa
            pa += 1
            a_ln[k] = pa
            pa += 1
            a_e[k] = pa

        # DVE order per step k: [g3 at k=10] z_k | upd_{k-1} | rcp_k | x_k
        # upd_{k-1} executes in the z->rcp gap (DVE idles ~135ns there
        # waiting on the ACT chain), keeping its wide op off the critical
        # path. [g2 before upd_2, i.e. in step 3's slot.]  Step D-1 has no
        # update, so its z^2 runs on the DVE instead of ACT Square — one
        # fewer op on the saturated ACT sequencer right before the output.
        d_z, d_z2, d_rcp, d_x, d_upd = {}, {}, {}, {}, {}
        pd = 0
        for k in range(D):
            if k == 10:
                pd += 1  # g3: DMA3 gate (yzB + lnB + WT_10..)
            pd += 1
            d_z[k] = pd
            if k == D - 1:
                pd += 1
                d_z2[k] = pd
            if 1 <= k <= D - 2:
                if k == 3:
                    pd += 1  # g2: DMA2 gate (WT_2..WT_9)
                pd += 1
                d_upd[k - 1] = pd
            pd += 1
            d_rcp[k] = pd
            pd += 1
            d_x[k] = pd

        @block.scalar
        def _(scalar):
            for k in range(D):
                if k >= 1:
                    nc.scalar.activation(
                        out=tt[:, :], in_=vx[:, k - 1:k], func=Act.Tanh,
                        bias=P[:, k:k + 1],
                        scale=float(Wq[k, k - 1]))._wait_ge(
                            sV, d_x[k - 1]).then_inc(sA, 1)
                # z2 = (scale*T + yz)^2 ; k=0: z_0 = yz_0 + c0
                if k == 0:
                    nc.scalar.activation(
                        out=z2[:, :], in_=yzc(0), func=Act.Square,
                        bias=c0c[:, :], scale=1.0)._wait_ge(
                            sV, d_z[0]).then_inc(sA, 1)
                elif k != D - 1:
                    nc.scalar.activation(
                        out=z2[:, :], in_=tt[:, :], func=Act.Square,
                        bias=yzc(k), scale=float(-kz[k]))._wait_ge(
                            sV, d_z[k]).then_inc(sA, 1)
                inst = nc.scalar.activation(
                    out=rr[:, :], in_=z2[:, :], func=Act.Sqrt,
                    bias=1.0, scale=1.0)
                if k == D - 1:
                    inst._wait_ge(sV, d_z2[k])
                else:
                    inst._wait_ge(sA, a_sq[k])
                inst.then_inc(sA, 1)
                nc.scalar.activation(
                    out=ll[:, :], in_=rr[:, :], func=Act.Ln, bias=zz[:, :],
                    scale=1.0)._wait_ge(sA, a_sqrt[k]).then_inc(sA, 1)
                nc.scalar.activation(
                    out=ee[:, :], in_=ll[:, :], func=Act.Exp,
                    bias=lnc(k), scale=float(1.0 / 3.0))._wait_ge(
                        sA, a_ln[k]).then_inc(sA, 1)

        @block.vector
        def _(vector):
            for k in range(D):
                if k == 10:
                    # DMA3 gate: z/Square k>=10 (yzB) and upd k>=10 (WT)
                    nc.vector.memset(scr[:, 0:1], 0.0)._wait_ge(
                        s_dma, 48).then_inc(sV, 1)
                # z_k = -kz*T + yz  (k=0: yz_0 + c0)
                if k == 0:
                    nc.vector.tensor_scalar(
                        out=zz[:, :], in0=yzc(0), scalar1=c0,
                        scalar2=None, op0=Alu.add)._wait_ge(
                            s_dma, 16).then_inc(sV, 1)
                else:
                    nc.vector.tensor_scalar(
                        out=zz[:, :], in0=tt[:, :],
                        scalar1=float(-kz[k]), scalar2=yzc(k),
                        op0=Alu.mult, op1=Alu.add)._wait_ge(
                            sA, a_tanh[k]).then_inc(sV, 1)
                if k == D - 1:
                    # last step's z^2 on DVE (no update here, so the DVE has
                    # a free slot; saves one op on the saturated ACT SEQ)
                    nc.vector.tensor_tensor(
                        out=z2[:, :], in0=zz[:, :], in1=zz[:, :],
                        op=Alu.mult)._wait_ge(
                            sV, d_z[k]).then_inc(sV, 1)
                # P[:, j+2:] += W[j+2:, j] * x_j  for j = k-1 (in z->rcp gap)
                if 1 <= k <= D - 2:
                    j = k - 1
                    if k == 3:
                        # DMA2 gate: WT_2..WT_9
                        nc.vector.memset(scr[:, 0:1], 0.0)._wait_ge(
                            s_dma, 32).then_inc(sV, 1)
                    nc.vector.scalar_tensor_tensor(
                        out=P[:, j + 2:D],
                        in0=hdr[:, wt_off[j]:wt_off[j] + wt_w[j]],
                        scalar=vx[:, j:j + 1],
                        in1=P[:, j + 2:D],
                        op0=Alu.mult, op1=Alu.add)._wait_ge(
                            sV, d_x[j]).then_inc(sV, 1)
                nc.vector.reciprocal(out=rcp[:, :], in_=ee[:, :])._wait_ge(
                    sA, a_e[k]).then_inc(sV, 1)
                # x_k = -sat^2*rcp + E'
                nc.vector.tensor_scalar(
                    out=vx[:, k:k + 1], in0=rcp[:, :],
                    scalar1=float(-sat2[k]), scalar2=ee[:, 0:1],
                    op0=Alu.mult, op1=Alu.add)._wait_ge(
                        sV, d_rcp[k]).then_inc(sV, 1)

        @block.sync
        def _(sync):
            sync.dma_start(out=hdr[:, 0:C1],
                           in_=hd_d[:, 0:C1]).then_inc(s_dma, 16)
            sync.dma_start(out=hdr[:, C1:C2],
                           in_=hd_d[:, C1:C2]).then_inc(s_dma, 16)
            sync.dma_start(out=hdr[:, C2:HW],
                           in_=hd_d[:, C2:HW]).then_inc(s_dma, 16)
            sync.dma_start(out=xo_d[:, :], in_=vx[:, :])._wait_ge(
                sV, d_x[D - 1]).then_inc(s_out, 16)
            sync.wait_ge(s_dma, 48)
            sync.wait_ge(s_out, 16)

    in_maps = []
    for c_ in range(NCORES):
        yzc_ = Yz[c_ * ROWS:(c_ + 1) * ROWS]
        in_maps.append({"hdr": hdr_np_for(yzc_)})
    return nc, in_maps


def kernel(y, W, s, b):
    from concourse.bass_utils import run_bass_kernel_spmd

    nc, in_maps = build(y, W, s, b)
    res = run_bass_kernel_spmd(nc, in_maps, list(range(NCORES))).results
    X = np.concatenate([res[c]["xout"] for c in range(NCORES)], axis=0)
    return X.astype(np.float32)


if __name__ == "__main__":
    rng = np.random.default_rng(0)
    y = rng.standard_normal((B, D)).astype(np.float32)
    W = np.tril(rng.standard_normal((32, 32)), -1).astype(np.float32) * 0.5
    s = rng.standard_normal(D).astype(np.float32)
    b = rng.standard_normal(D).astype(np.float32)
    X = kernel(y=y, W=W, s=s, b=b)
    print("out", X.shape, X.dtype, X[0, :4])
